# revision 25
# baseline (speedup 1.0000x reference)
"""EnsembleUncertaintyDistance Trainium2 kernel, v2.

out[q,p] = mean_m(o_mqp) * exp(-std_m(o_mqp)),
o_mqp = sum_h W2[m,h]*relu(hq[m,q,h]+hp[m,p,h]+b1[m,h]) + b2[m]

v2 changes vs v1 (same math, same host folding of |W2| signs):
  - N=512 main matmuls: each MM consumes two produced tiles (q, q+16)
    side by side in the free dim -> half the PE instructions.
  - producers write 8-slot superbuffers (one tile acquisition per 8
    producer ops) -> fewer semaphore waits on the DVE/ACT queues.
  - tail avoids ACT Ln entirely (fast rsqrt on DVE via the bit trick)
    so the only ACT table set ever loaded is exp_and_others
    (exp+relu+square+identity) -> no mid-kernel ACT_TABLE_LOAD thrash.
  - input DMA chunked along the EMB contraction so the hq/hp prep
    matmuls start after the first chunk lands.

v6 changes vs v2:
  - deferred emission: engine queues are in-order, so the serial
    drain/stats/tail chains used to head-of-line-block the produce
    stream at each half/rep boundary.  Drains/stats/tail are emitted
    1/3/4+ produce-chunks late, and the tail is split at each
    cross-engine hop (DVE newton -> ACT exp -> DVE res) so every
    segment's dependency is complete before it reaches a queue head.
  - the tail runs in bf16/int16 (magic 0x5F37) -> DVE 2x packed mode,
    shorter serial chain; rsqrt error 3.7% vs 3.4% for the f32 magic.
  - PE warm-keeper throwaway matmuls dropped: the PE now streams the
    next rep's mains across the boundary instead.
"""
import threading
import time

import numpy as np
import ml_dtypes

EMB = 512
NM = 5
HID = 64
NQ = 2048
NP = 256
NCORES = 8
NQL = NQ // NCORES  # 256 local q rows
P = 128

MAGIC = 0x5F3759DF
MAGIC16 = 0x5F37

_CACHE = {}
_LOCK = threading.Lock()


def _build_nc(repeat=1, act_every=3, tail_mode="newton", newton_iters=0,
              act_skim=22):
    import concourse.mybir as mybir
    import concourse.tile as tile
    from concourse import bacc

    f32 = mybir.dt.float32
    i32 = mybir.dt.int32
    i16 = mybir.dt.int16
    bf16 = mybir.dt.bfloat16
    ADD = mybir.AluOpType.add
    MULT = mybir.AluOpType.mult
    SUB = mybir.AluOpType.subtract
    MAX = mybir.AluOpType.max
    SHR = mybir.AluOpType.logical_shift_right
    RELU = mybir.ActivationFunctionType.Relu
    EXP = mybir.ActivationFunctionType.Exp
    IDENT = mybir.ActivationFunctionType.Identity
    SQUARE = mybir.ActivationFunctionType.Square
    LN = mybir.ActivationFunctionType.Ln

    nc = bacc.Bacc("TRN2", target_bir_lowering=False, debug=False,
                   num_devices=NCORES)

    qT = nc.dram_tensor("qT", [EMB, NQL], f32, kind="ExternalInput").ap()
    pT = nc.dram_tensor("pT", [EMB, NP], f32, kind="ExternalInput").ap()
    w1qT = nc.dram_tensor("w1qT", [EMB, NM * HID], f32, kind="ExternalInput").ap()
    w1pT = nc.dram_tensor("w1pT", [EMB, NM * HID], f32, kind="ExternalInput").ap()
    b1c = nc.dram_tensor("b1c", [P, 3], f32, kind="ExternalInput").ap()
    wsel = nc.dram_tensor("wsel", [P, 3 * 16 * 32], bf16, kind="ExternalInput").ap()
    b2c = nc.dram_tensor("b2c", [P, 3], f32, kind="ExternalInput").ap()
    smat = nc.dram_tensor("smat", [P, 512], bf16, kind="ExternalInput").ap()
    out = nc.dram_tensor("out", [NQL, NP], f32, kind="ExternalOutput").ap()

    with tile.TileContext(nc) as tc:
        with tc.tile_pool(name="const", bufs=1) as cpool, \
             tc.tile_pool(name="sbp", bufs=4) as sbp, \
             tc.tile_pool(name="spool", bufs=2) as spool, \
             tc.tile_pool(name="mps", bufs=2, space="PSUM") as mps:

            # ---- SBUF input tiles ----
            qT_sb = cpool.tile([P, EMB // P, NQL], f32)
            pT_sb = cpool.tile([P, EMB // P, NP], f32)
            w1qT_sb = cpool.tile([P, EMB // P, NM * HID], f32)
            w1pT_sb = cpool.tile([P, EMB // P, NM * HID], f32)
            b1c_sb = cpool.tile([P, 3], f32)
            wsel_sb = cpool.tile([P, 3, 16, 32], bf16)
            b2c_sb = cpool.tile([P, 3], f32)
            smat_sb = cpool.tile([P, 4, P], bf16)

            qT_r = qT.rearrange("(o p) q -> p o q", p=P)
            pT_r = pT.rearrange("(o p) q -> p o q", p=P)
            w1qT_r = w1qT.rearrange("(o p) q -> p o q", p=P)
            w1pT_r = w1pT.rearrange("(o p) q -> p o q", p=P)
            # chunked input DMA: hq-path chunk k first so prep can start
            for k in range(EMB // P):
                nc.sync.dma_start(w1qT_sb[:, k, :], w1qT_r[:, k, :])
                nc.sync.dma_start(qT_sb[:, k, :], qT_r[:, k, :])
                nc.sync.dma_start(w1pT_sb[:, k, :], w1pT_r[:, k, :])
                nc.sync.dma_start(pT_sb[:, k, :], pT_r[:, k, :])
            nc.sync.dma_start(b1c_sb[:], b1c)
            nc.sync.dma_start(wsel_sb[:], wsel.rearrange("p (a u c) -> p a u c", a=3, u=16))
            nc.sync.dma_start(b2c_sb[:], b2c)
            nc.sync.dma_start(smat_sb[:], smat.rearrange("p (a c) -> p a c", a=4))

            # ACT warm-up: touch only the exp_and_others set (exp + relu +
            # square fillers).  Nothing else loads a table mid-kernel.
            scr = cpool.tile([1, 2], f32)
            nc.vector.memset(scr[:], 1.0)
            ln02 = cpool.tile([P, 1], f32)
            nc.vector.memset(ln02[:], float(np.log(0.2)))
            nc.scalar.activation(scr[:, 0:1], scr[:, 0:1], EXP)
            nc.scalar.activation(scr[:, 0:1], scr[:, 0:1], RELU)
            if tail_mode == "lnexp":
                nc.scalar.activation(scr[:, 0:1], scr[:, 0:1], LN)

            # ---- hq/hp prep matmuls (k-chunk-major, overlaps input DMA) ----
            hq_ps01 = mps.tile([P, 2, NQL], f32, tag="b01")
            hq_ps23 = mps.tile([P, 2, NQL], f32, tag="b23")
            hp_ps01 = mps.tile([P, 2, NP], f32, tag="b44")
            hp_ps23 = mps.tile([P, 2, NP], f32, tag="stats")

            regions = [
                (hq_ps01[0:HID, 0, :], w1qT_sb, qT_sb, 0),
                (hq_ps01[HID:P, 0, :], w1qT_sb, qT_sb, 1),
                (hq_ps23[0:HID, 0, :], w1qT_sb, qT_sb, 2),
                (hq_ps23[HID:P, 0, :], w1qT_sb, qT_sb, 3),
                (hq_ps01[0:HID, 1, :], w1qT_sb, qT_sb, 4),
                (hq_ps01[HID:P, 1, :], w1qT_sb, qT_sb, 4),
                (hp_ps01[0:HID, 0, :], w1pT_sb, pT_sb, 0),
                (hp_ps01[HID:P, 0, :], w1pT_sb, pT_sb, 1),
                (hp_ps23[0:HID, 0, :], w1pT_sb, pT_sb, 2),
                (hp_ps23[HID:P, 0, :], w1pT_sb, pT_sb, 3),
                (hp_ps01[0:HID, 1, :], w1pT_sb, pT_sb, 4),
                (hp_ps01[HID:P, 1, :], w1pT_sb, pT_sb, 4),
            ]
            # PSUM start=True lazily zero-marks the full partition-rows of
            # the bank, so only partition-disjoint groups may interleave.
            # Regions are processed in pairs ([0:64] with [64:128]); pairs
            # sharing a bank stay strictly sequential.
            for pair in range(6):
                for k in range(EMB // P):
                    for (ps_out, w_sb, x_sb, m) in regions[2 * pair:2 * pair + 2]:
                        nc.tensor.matmul(
                            ps_out, w_sb[:, k, m * HID:(m + 1) * HID],
                            x_sb[:, k, :],
                            start=(k == 0), stop=(k == EMB // P - 1),
                            skip_group_check=True)

            # ---- producer operands in SBUF ----
            hq01 = cpool.tile([P, NQL], f32)
            hq23 = cpool.tile([P, NQL], f32)
            hq4p = cpool.tile([P, NQL // 2], f32)
            hpb01 = cpool.tile([P, NP], bf16)
            hpb23 = cpool.tile([P, NP], bf16)
            hpb4d = cpool.tile([P, NP], bf16)

            nc.vector.tensor_copy(out=hq01[:], in_=hq_ps01[:, 0, :])
            nc.vector.tensor_copy(out=hq23[:], in_=hq_ps23[:, 0, :])
            hq4_pairs = hq_ps01[:, 1, :].rearrange("p (q two) -> p two q", two=2)
            nc.vector.tensor_copy(out=hq4p[0:HID, :], in_=hq4_pairs[0:HID, 0, :])
            nc.vector.tensor_copy(out=hq4p[HID:P, :], in_=hq4_pairs[HID:P, 1, :])

            nc.vector.tensor_scalar(hpb01[:], hp_ps01[:, 0, :], b1c_sb[:, 0:1],
                                    None, ADD)
            nc.vector.tensor_scalar(hpb23[:], hp_ps23[:, 0, :], b1c_sb[:, 1:2],
                                    None, ADD)
            nc.vector.tensor_scalar(hpb4d[:], hp_ps01[:, 1, :], b1c_sb[:, 2:3],
                                    None, ADD)

            tile_idx = 0
            act_n = 0

            def produce(dst, hq_t, col):
                nonlocal tile_idx, act_n
                i = tile_idx
                tile_idx += 1
                hpb_t = {id(hq01): hpb01, id(hq23): hpb23,
                         id(hq4p): hpb4d}[id(hq_t)]
                on_act = i % act_every == act_every - 1
                if on_act and act_skim:
                    # ACT is the tighter queue; skim every act_skim-th ACT
                    # assignment back to DVE to balance
                    act_n += 1
                    if act_n % act_skim == 0:
                        on_act = False
                if on_act:
                    nc.scalar.activation(dst, hpb_t[:], RELU,
                                         bias=hq_t[:, col:col + 1])
                else:
                    nc.vector.tensor_scalar(dst, hpb_t[:],
                                            hq_t[:, col:col + 1], 0.0,
                                            ADD, MAX)

            # ---- deferred-emission schedule ----
            # Engine queues are in-order; the serial drain/stats/tail
            # chains would head-of-line-block the produce stream at every
            # half/rep boundary.  Emit them a few produce-chunks late, and
            # split the tail at each cross-engine hop (DVE newton -> ACT
            # exp -> DVE res) so neither queue ever stalls at its head:
            # by the time each segment surfaces, its dependency is done.
            banks = {}    # (rep, half) -> (bank01, bank23, bank44)
            osqs = {}     # (rep, half) -> (o01, o23, o44, sq01, sq23, sq44)
            psum5s = {}   # rep -> {half: psum5}
            tails = {}    # rep -> dict of tail tiles

            def mm_chunk(rep, half, g, chunk):
                qoff0 = P * half
                if g == 0 and chunk == 0:
                    banks[(rep, half)] = (
                        mps.tile([P, 2, NP], f32, tag="b01", name="bank01"),
                        mps.tile([P, 2, NP], f32, tag="b23", name="bank23"),
                        mps.tile([P, 2, NP], f32, tag="b44", name="bank44"))
                bank01, bank23, bank44 = banks[(rep, half)]
                # pair01 superbuffer: 4 MMs (u = 4c..4c+3)
                sb01 = sbp.tile([P, 8, NP], bf16, tag="sb01")
                for j, u in enumerate(range(4 * chunk, 4 * chunk + 4)):
                    for c in range(2):
                        qq = qoff0 + 32 * g + 16 * c + u
                        produce(sb01[:, 2 * j + c, :], hq01, qq)
                for j, u in enumerate(range(4 * chunk, 4 * chunk + 4)):
                    nc.tensor.matmul(
                        bank01[32 * g:32 * (g + 1), :, :],
                        wsel_sb[:, 0, u, :],
                        sb01[:, 2 * j:2 * j + 2, :],
                        start=(u == 0), stop=(u == 15),
                        tile_position=(0, 32 * g),
                        skip_group_check=True)
                # pair23
                sb23 = sbp.tile([P, 8, NP], bf16, tag="sb23")
                for j, u in enumerate(range(4 * chunk, 4 * chunk + 4)):
                    for c in range(2):
                        qq = qoff0 + 32 * g + 16 * c + u
                        produce(sb23[:, 2 * j + c, :], hq23, qq)
                for j, u in enumerate(range(4 * chunk, 4 * chunk + 4)):
                    nc.tensor.matmul(
                        bank23[32 * g:32 * (g + 1), :, :],
                        wsel_sb[:, 1, u, :],
                        sb23[:, 2 * j:2 * j + 2, :],
                        start=(u == 0), stop=(u == 15),
                        tile_position=(0, 32 * g),
                        skip_group_check=True)
                # model-4: 8 MMs per g -> on chunks 0 and 2
                if chunk % 2 == 0:
                    vc = chunk // 2
                    sb44 = sbp.tile([P, 8, NP], bf16, tag="sb44")
                    for j, v in enumerate(range(4 * vc, 4 * vc + 4)):
                        for c in range(2):
                            w = 64 * half + 16 * g + 8 * c + v
                            produce(sb44[:, 2 * j + c, :], hq4p, w)
                    for j, v in enumerate(range(4 * vc, 4 * vc + 4)):
                        nc.tensor.matmul(
                            bank44[32 * g:32 * (g + 1), :, :],
                            wsel_sb[:, 2, v, :],
                            sb44[:, 2 * j:2 * j + 2, :],
                            start=(v == 0), stop=(v == 7),
                            tile_position=(0, 32 * g),
                            skip_group_check=True)

            def emit_drains(rep, half):
                bank01, bank23, bank44 = banks[(rep, half)]
                o01 = spool.tile([P, 2, NP], bf16, tag="o01")
                o23 = spool.tile([P, 2, NP], bf16, tag="o23")
                o44 = spool.tile([P, 2, NP], bf16, tag="o44")
                sq01 = spool.tile([P, 2, NP], bf16, tag="sq01")
                sq23 = spool.tile([P, 2, NP], bf16, tag="sq23")
                sq44 = spool.tile([P, 2, NP], bf16, tag="sq44")
                osqs[(rep, half)] = (o01, o23, o44, sq01, sq23, sq44)
                # drains (+b2) all on ACT: the DVE queue is the binding
                # resource; ACT has slack
                nc.scalar.activation(o01[:], bank01[:], IDENT,
                                     bias=b2c_sb[:, 0:1])
                nc.scalar.activation(o23[:], bank23[:], IDENT,
                                     bias=b2c_sb[:, 1:2])
                nc.scalar.activation(o44[:], bank44[:], IDENT,
                                     bias=b2c_sb[:, 2:3])
                # squares: 01/23 on DVE (bf16 TT), 44 on ACT
                nc.vector.tensor_tensor(sq01[:], o01[:], o01[:], MULT)
                nc.vector.tensor_tensor(sq23[:], o23[:], o23[:], MULT)
                nc.scalar.activation(sq44[:], o44[:], SQUARE)

            def emit_stats(rep, half):
                # model sums / sums of squares via PE
                psum5 = mps.tile([P, 2, NP], f32, tag="stats")
                psum5s.setdefault(rep, {})[half] = psum5
                o01, o23, o44, sq01, sq23, sq44 = osqs[(rep, half)]
                for dst, srcs in ((psum5[:, 0, :], (o01, o23, o44)),
                                  (psum5[:, 1, :], (sq01, sq23, sq44))):
                    first = True
                    for c in range(2):
                        nc.tensor.matmul(dst, smat_sb[:, c, :],
                                         srcs[0][:, c, :],
                                         start=first, stop=False)
                        first = False
                        nc.tensor.matmul(dst, smat_sb[:, c, :],
                                         srcs[1][:, c, :],
                                         start=False, stop=False)
                        nc.tensor.matmul(dst, smat_sb[:, 2 + c, :],
                                         srcs[2][:, c, :],
                                         start=False, stop=(c == 1))

            def emit_tail_a(rep):
                # d = ss5 - 0.2*s5^2 per half, then bf16 fast-rsqrt (all
                # DVE; int16/bf16 keeps the 2x packed mode)
                p5 = psum5s[rep]
                d_all = spool.tile([P, 2, NP], bf16, tag="d_all")
                sd = spool.tile([P, 2, NP], bf16, tag="sd")
                tails[rep] = {"p5": p5, "sd": sd}
                for half in range(2):
                    su2 = spool.tile([P, NP], f32, tag="su2")
                    s2a = spool.tile([P, NP], f32, tag="s2a")
                    nc.scalar.activation(su2[:], p5[half][:, 0, :],
                                         IDENT, scale=float(np.sqrt(0.2)))
                    nc.scalar.activation(s2a[:], su2[:], SQUARE)
                    nc.vector.tensor_tensor(d_all[:, half, :],
                                            p5[half][:, 1, :],
                                            s2a[:], SUB)
                ti = spool.tile([P, 2, NP], i16, tag="ti")
                y0i = spool.tile([P, 2, NP], i16, tag="y0i")
                d_i = d_all[:].bitcast(i16)
                nc.vector.tensor_scalar(ti[:], d_i, 1, None, SHR)
                # MAGIC16 - t  ==  (t - MAGIC16) * -1
                nc.vector.tensor_scalar(y0i[:], ti[:], MAGIC16, -1.0,
                                        SUB, MULT)
                y0 = y0i[:].bitcast(bf16)
                nc.vector.tensor_tensor(sd[:], d_all[:], y0, MULT)

            def emit_tail_b(rep):
                # e2 = exp(-0.5*sd + ln(0.2)) = 0.2*exp(-std)   (ACT)
                e2 = spool.tile([P, 2, NP], f32, tag="e2")
                tails[rep]["e2"] = e2
                nc.scalar.activation(e2[:], tails[rep]["sd"][:], EXP,
                                     scale=-0.5, bias=ln02[:, 0:1])

            def emit_tail_c(rep):
                t = tails.pop(rep)
                p5, e2 = t["p5"], t["e2"]
                res_all = spool.tile([P, 2, NP], f32, tag="res_all")
                for half in range(2):
                    nc.vector.tensor_tensor(res_all[:, half, :],
                                            e2[:, half, :],
                                            p5[half][:, 0, :], MULT)
                    nc.sync.dma_start(out[P * half:P * (half + 1), :],
                                      res_all[:, half, :])

            # boundary b = number of mm chunks emitted before the event
            D_DRAIN, D_STATS = 1, 3
            D_TA, D_TB, D_TC = 5, 6, 7
            boundary = {}
            for rep in range(repeat):
                for half in range(2):
                    end = 32 * rep + 16 * half + 16
                    boundary.setdefault(end + D_DRAIN, []).append(
                        ("drain", rep, half))
                    boundary.setdefault(end + D_STATS, []).append(
                        ("stats", rep, half))
                end = 32 * rep + 32
                boundary.setdefault(end + D_TA, []).append(("ta", rep))
                boundary.setdefault(end + D_TB, []).append(("tb", rep))
                boundary.setdefault(end + D_TC, []).append(("tc", rep))
            fns = {"drain": emit_drains, "stats": emit_stats,
                   "ta": emit_tail_a, "tb": emit_tail_b, "tc": emit_tail_c}

            count = 0
            for rep in range(repeat):
                for half in range(2):
                    for g in range(4):
                        for chunk in range(4):
                            mm_chunk(rep, half, g, chunk)
                            count += 1
                            for kind, *a in boundary.pop(count, []):
                                fns[kind](*a)
            for b in sorted(boundary):
                for kind, *a in boundary[b]:
                    fns[kind](*a)

    nc.compile()
    return nc


class _Runner:
    def __init__(self, nc, n_cores=NCORES):
        import jax
        from jax.sharding import Mesh, PartitionSpec
        from jax.experimental.shard_map import shard_map
        import concourse.mybir as mybir
        from concourse import bass2jax

        bass2jax.install_neuronx_cc_hook()
        self.jax = jax
        self.n_cores = n_cores
        self.in_names, self.out_names, out_avals, self.zero_outs = [], [], [], []
        pname = nc.partition_id_tensor.name if nc.partition_id_tensor else None
        for alloc in nc.m.functions[0].allocations:
            if not isinstance(alloc, mybir.MemoryLocationSet):
                continue
            name = alloc.memorylocations[0].name
            if alloc.kind == "ExternalInput":
                if name != pname:
                    self.in_names.append(name)
            elif alloc.kind == "ExternalOutput":
                self.out_names.append(name)
                shape = tuple(alloc.tensor_shape)
                dtype = mybir.dt.np(alloc.dtype)
                out_avals.append(jax.core.ShapedArray(shape, dtype))
                self.zero_outs.append(np.zeros(shape, dtype))
        n_params = len(self.in_names)
        n_outs = len(out_avals)
        all_names = tuple(self.in_names + self.out_names + ([pname] if pname else []))
        out_names = tuple(self.out_names)

        def _body(*args):
            operands = list(args)
            if pname is not None:
                operands.append(bass2jax.partition_id_tensor())
            return tuple(bass2jax._bass_exec_p.bind(
                *operands, out_avals=tuple(out_avals), in_names=all_names,
                out_names=out_names, lowering_input_output_aliases=(),
                sim_require_finite=True, sim_require_nnan=True, nc=nc))

        devices = jax.devices()[:n_cores]
        mesh = Mesh(np.asarray(devices), ("core",))
        self.fn = jax.jit(
            shard_map(_body, mesh=mesh,
                      in_specs=(PartitionSpec("core"),) * (n_params + n_outs),
                      out_specs=(PartitionSpec("core"),) * n_outs,
                      check_rep=False),
            keep_unused=True)

    def concat_inputs(self, in_maps):
        cat = [np.concatenate([np.asarray(m[name]) for m in in_maps], axis=0)
               for name in self.in_names]
        cat += [np.zeros((self.n_cores * z.shape[0], *z.shape[1:]), z.dtype)
                for z in self.zero_outs]
        return cat

    def run(self, in_maps):
        outs = self.fn(*self.concat_inputs(in_maps))
        self.jax.block_until_ready(outs)
        res = []
        for c in range(self.n_cores):
            d = {}
            for i, name in enumerate(self.out_names):
                full = np.asarray(outs[i])
                per = full.shape[0] // self.n_cores
                d[name] = full[c * per:(c + 1) * per]
            res.append(d)
        return res

    def time_it(self, in_maps, iters=20):
        args = [self.jax.device_put(x) for x in self.concat_inputs(in_maps)]
        outs = self.fn(*args)
        self.jax.block_until_ready(outs)
        times = []
        for _ in range(iters):
            t0 = time.perf_counter()
            outs = self.fn(*args)
            self.jax.block_until_ready(outs)
            times.append((time.perf_counter() - t0) * 1e9)
        times.sort()
        return times


def _host_prep(query_features, prototypes, W1, b1, W2, b2):
    """Host-side layout prep; returns per-core in_maps."""
    q = np.asarray(query_features, np.float32)
    p = np.asarray(prototypes, np.float32)
    W1 = np.asarray(W1, np.float32)
    b1 = np.asarray(b1, np.float32)
    W2 = np.asarray(W2, np.float32)
    b2 = np.asarray(b2, np.float32)

    absW2 = np.abs(W2)                       # [M, H]
    sgnW2 = np.where(W2 >= 0, 1.0, -1.0).astype(np.float32)
    W1q = W1[:, :, :EMB] * absW2[:, :, None]  # [M, H, E]
    W1p = W1[:, :, EMB:] * absW2[:, :, None]
    b1s = b1 * absW2                         # [M, H]

    qT = np.ascontiguousarray(q.T)           # [E, NQ]
    pT = np.ascontiguousarray(p.T)           # [E, NP]
    w1qT = np.ascontiguousarray(W1q.transpose(2, 0, 1).reshape(EMB, NM * HID))
    w1pT = np.ascontiguousarray(W1p.transpose(2, 0, 1).reshape(EMB, NM * HID))

    b1c = np.zeros((P, 3), np.float32)
    b1c[:HID, 0], b1c[HID:, 0] = b1s[0], b1s[1]
    b1c[:HID, 1], b1c[HID:, 1] = b1s[2], b1s[3]
    b1c[:HID, 2], b1c[HID:, 2] = b1s[4], b1s[4]

    wsel = np.zeros((P, 3, 16, 32), np.float32)
    model_pairs = ((0, 1), (2, 3), (4, 4))
    for pi, (ma, mb) in enumerate(model_pairs):
        for u in range(16):
            wsel[:HID, pi, u, 2 * u] = sgnW2[ma]
            wsel[HID:, pi, u, 2 * u + 1] = sgnW2[mb]
    wsel = wsel.reshape(P, 3 * 16 * 32).astype(ml_dtypes.bfloat16)

    b2c = np.zeros((P, 3), np.float32)
    b2c[0::2, 0], b2c[1::2, 0] = b2[0], b2[1]
    b2c[0::2, 1], b2c[1::2, 1] = b2[2], b2[3]
    b2c[:, 2] = b2[4]

    # smat: [P01_c0 | P01_c1 | P44_c0 | P44_c1], each [128, 128]
    smat = np.zeros((P, 4, P), np.float32)
    for c in range(2):
        for g in range(4):
            for u in range(16):
                for m in range(2):
                    smat[32 * g + 2 * u + m, c, 32 * g + 16 * c + u] = 1.0
            for v in range(8):
                for par in range(2):
                    smat[32 * g + 2 * v + par, 2 + c,
                         32 * g + 16 * c + 2 * v + par] = 1.0
    smat = smat.reshape(P, 4 * P).astype(ml_dtypes.bfloat16)

    shared = dict(pT=pT, w1qT=w1qT, w1pT=w1pT, b1c=b1c, wsel=wsel,
                  b2c=b2c, smat=smat)
    in_maps = []
    for c in range(NCORES):
        m = dict(shared)
        m["qT"] = np.ascontiguousarray(qT[:, c * NQL:(c + 1) * NQL])
        in_maps.append(m)
    return in_maps


def _get_runner(repeat=1, act_every=3, tail_mode="newton"):
    key = (repeat, act_every, tail_mode)
    with _LOCK:
        if key not in _CACHE:
            nc = _build_nc(repeat=repeat, act_every=act_every,
                           tail_mode=tail_mode)
            _CACHE[key] = _Runner(nc)
        return _CACHE[key]


def kernel(query_features, prototypes, W1, b1, W2, b2):
    in_maps = _host_prep(query_features, prototypes, W1, b1, W2, b2)
    runner = _get_runner()
    res = runner.run(in_maps)
    return np.concatenate([res[c]["out"] for c in range(NCORES)], axis=0)



# revision 27
# speedup vs baseline: 1.0225x; 1.0225x over previous
"""EnsembleUncertaintyDistance Trainium2 kernel, v2.

out[q,p] = mean_m(o_mqp) * exp(-std_m(o_mqp)),
o_mqp = sum_h W2[m,h]*relu(hq[m,q,h]+hp[m,p,h]+b1[m,h]) + b2[m]

v2 changes vs v1 (same math, same host folding of |W2| signs):
  - N=512 main matmuls: each MM consumes two produced tiles (q, q+16)
    side by side in the free dim -> half the PE instructions.
  - producers write 8-slot superbuffers (one tile acquisition per 8
    producer ops) -> fewer semaphore waits on the DVE/ACT queues.
  - tail avoids ACT Ln entirely (fast rsqrt on DVE via the bit trick)
    so the only ACT table set ever loaded is exp_and_others
    (exp+relu+square+identity) -> no mid-kernel ACT_TABLE_LOAD thrash.
  - input DMA chunked along the EMB contraction so the hq/hp prep
    matmuls start after the first chunk lands.

v6 changes vs v2:
  - deferred emission: engine queues are in-order, so the serial
    drain/stats/tail chains used to head-of-line-block the produce
    stream at each half/rep boundary.  Drains/stats/tail are emitted
    1/3/4+ produce-chunks late, and the tail is split at each
    cross-engine hop (DVE newton -> ACT exp -> DVE res) so every
    segment's dependency is complete before it reaches a queue head.
  - the tail runs in bf16/int16 (magic 0x5F37) -> DVE 2x packed mode,
    shorter serial chain; rsqrt error 3.7% vs 3.4% for the f32 magic.
  - PE warm-keeper throwaway matmuls dropped: the PE now streams the
    next rep's mains across the boundary instead.
"""
import threading
import time

import numpy as np
import ml_dtypes

EMB = 512
NM = 5
HID = 64
NQ = 2048
NP = 256
NCORES = 8
NQL = NQ // NCORES  # 256 local q rows
P = 128

MAGIC = 0x5F3759DF
MAGIC16 = 0x5F37

_CACHE = {}
_LOCK = threading.Lock()


def _build_nc(repeat=1, act_every=3, tail_mode="newton", newton_iters=0,
              act_skim=32):
    import concourse.mybir as mybir
    import concourse.tile as tile
    from concourse import bacc

    f32 = mybir.dt.float32
    i32 = mybir.dt.int32
    i16 = mybir.dt.int16
    bf16 = mybir.dt.bfloat16
    ADD = mybir.AluOpType.add
    MULT = mybir.AluOpType.mult
    SUB = mybir.AluOpType.subtract
    MAX = mybir.AluOpType.max
    SHR = mybir.AluOpType.logical_shift_right
    RELU = mybir.ActivationFunctionType.Relu
    EXP = mybir.ActivationFunctionType.Exp
    IDENT = mybir.ActivationFunctionType.Identity
    SQUARE = mybir.ActivationFunctionType.Square
    LN = mybir.ActivationFunctionType.Ln

    nc = bacc.Bacc("TRN2", target_bir_lowering=False, debug=False,
                   num_devices=NCORES)

    qT = nc.dram_tensor("qT", [EMB, NQL], f32, kind="ExternalInput").ap()
    pT = nc.dram_tensor("pT", [EMB, NP], f32, kind="ExternalInput").ap()
    w1qT = nc.dram_tensor("w1qT", [EMB, NM * HID], f32, kind="ExternalInput").ap()
    w1pT = nc.dram_tensor("w1pT", [EMB, NM * HID], f32, kind="ExternalInput").ap()
    b1c = nc.dram_tensor("b1c", [P, 3], f32, kind="ExternalInput").ap()
    wsel = nc.dram_tensor("wsel", [P, 3 * 16 * 32], bf16, kind="ExternalInput").ap()
    b2c = nc.dram_tensor("b2c", [P, 3], f32, kind="ExternalInput").ap()
    smat = nc.dram_tensor("smat", [P, 512], bf16, kind="ExternalInput").ap()
    out = nc.dram_tensor("out", [NQL, NP], f32, kind="ExternalOutput").ap()

    with tile.TileContext(nc) as tc:
        with tc.tile_pool(name="const", bufs=1) as cpool, \
             tc.tile_pool(name="sbp", bufs=4) as sbp, \
             tc.tile_pool(name="spool", bufs=2) as spool, \
             tc.tile_pool(name="mps", bufs=2, space="PSUM") as mps:

            # ---- SBUF input tiles ----
            qT_sb = cpool.tile([P, EMB // P, NQL], f32)
            pT_sb = cpool.tile([P, EMB // P, NP], f32)
            w1qT_sb = cpool.tile([P, EMB // P, NM * HID], f32)
            w1pT_sb = cpool.tile([P, EMB // P, NM * HID], f32)
            b1c_sb = cpool.tile([P, 3], f32)
            wsel_sb = cpool.tile([P, 3, 16, 32], bf16)
            b2c_sb = cpool.tile([P, 3], f32)
            smat_sb = cpool.tile([P, 4, P], bf16)

            qT_r = qT.rearrange("(o p) q -> p o q", p=P)
            pT_r = pT.rearrange("(o p) q -> p o q", p=P)
            w1qT_r = w1qT.rearrange("(o p) q -> p o q", p=P)
            w1pT_r = w1pT.rearrange("(o p) q -> p o q", p=P)
            # chunked input DMA: hq-path chunk k first so prep can start
            for k in range(EMB // P):
                nc.sync.dma_start(w1qT_sb[:, k, :], w1qT_r[:, k, :])
                nc.sync.dma_start(qT_sb[:, k, :], qT_r[:, k, :])
                nc.sync.dma_start(w1pT_sb[:, k, :], w1pT_r[:, k, :])
                nc.sync.dma_start(pT_sb[:, k, :], pT_r[:, k, :])
            nc.sync.dma_start(b1c_sb[:], b1c)
            nc.sync.dma_start(wsel_sb[:], wsel.rearrange("p (a u c) -> p a u c", a=3, u=16))
            nc.sync.dma_start(b2c_sb[:], b2c)
            nc.sync.dma_start(smat_sb[:], smat.rearrange("p (a c) -> p a c", a=4))

            # ACT warm-up: touch only the exp_and_others set (exp + relu +
            # square fillers).  Nothing else loads a table mid-kernel.
            scr = cpool.tile([1, 2], f32)
            nc.vector.memset(scr[:], 1.0)
            ln02 = cpool.tile([P, 1], f32)
            nc.vector.memset(ln02[:], float(np.log(0.2)))
            nc.scalar.activation(scr[:, 0:1], scr[:, 0:1], EXP)
            nc.scalar.activation(scr[:, 0:1], scr[:, 0:1], RELU)
            if tail_mode == "lnexp":
                nc.scalar.activation(scr[:, 0:1], scr[:, 0:1], LN)

            # ---- hq/hp prep matmuls (k-chunk-major, overlaps input DMA) ----
            hq_ps01 = mps.tile([P, 2, NQL], f32, tag="b01")
            hq_ps23 = mps.tile([P, 2, NQL], f32, tag="b23")
            hp_ps01 = mps.tile([P, 2, NP], f32, tag="b44")
            hp_ps23 = mps.tile([P, 2, NP], f32, tag="stats")

            regions = [
                (hq_ps01[0:HID, 0, :], w1qT_sb, qT_sb, 0),
                (hq_ps01[HID:P, 0, :], w1qT_sb, qT_sb, 1),
                (hq_ps23[0:HID, 0, :], w1qT_sb, qT_sb, 2),
                (hq_ps23[HID:P, 0, :], w1qT_sb, qT_sb, 3),
                (hq_ps01[0:HID, 1, :], w1qT_sb, qT_sb, 4),
                (hq_ps01[HID:P, 1, :], w1qT_sb, qT_sb, 4),
                (hp_ps01[0:HID, 0, :], w1pT_sb, pT_sb, 0),
                (hp_ps01[HID:P, 0, :], w1pT_sb, pT_sb, 1),
                (hp_ps23[0:HID, 0, :], w1pT_sb, pT_sb, 2),
                (hp_ps23[HID:P, 0, :], w1pT_sb, pT_sb, 3),
                (hp_ps01[0:HID, 1, :], w1pT_sb, pT_sb, 4),
                (hp_ps01[HID:P, 1, :], w1pT_sb, pT_sb, 4),
            ]
            # PSUM start=True lazily zero-marks the full partition-rows of
            # the bank, so only partition-disjoint groups may interleave.
            # Regions are processed in pairs ([0:64] with [64:128]); pairs
            # sharing a bank stay strictly sequential.
            for pair in range(6):
                for k in range(EMB // P):
                    for (ps_out, w_sb, x_sb, m) in regions[2 * pair:2 * pair + 2]:
                        nc.tensor.matmul(
                            ps_out, w_sb[:, k, m * HID:(m + 1) * HID],
                            x_sb[:, k, :],
                            start=(k == 0), stop=(k == EMB // P - 1),
                            skip_group_check=True)

            # ---- producer operands in SBUF ----
            hq01 = cpool.tile([P, NQL], f32)
            hq23 = cpool.tile([P, NQL], f32)
            hq4p = cpool.tile([P, NQL // 2], f32)
            hpb01 = cpool.tile([P, NP], bf16)
            hpb23 = cpool.tile([P, NP], bf16)
            hpb4d = cpool.tile([P, NP], bf16)

            nc.vector.tensor_copy(out=hq01[:], in_=hq_ps01[:, 0, :])
            nc.vector.tensor_copy(out=hq23[:], in_=hq_ps23[:, 0, :])
            hq4_pairs = hq_ps01[:, 1, :].rearrange("p (q two) -> p two q", two=2)
            nc.vector.tensor_copy(out=hq4p[0:HID, :], in_=hq4_pairs[0:HID, 0, :])
            nc.vector.tensor_copy(out=hq4p[HID:P, :], in_=hq4_pairs[HID:P, 1, :])

            nc.vector.tensor_scalar(hpb01[:], hp_ps01[:, 0, :], b1c_sb[:, 0:1],
                                    None, ADD)
            nc.vector.tensor_scalar(hpb23[:], hp_ps23[:, 0, :], b1c_sb[:, 1:2],
                                    None, ADD)
            nc.vector.tensor_scalar(hpb4d[:], hp_ps01[:, 1, :], b1c_sb[:, 2:3],
                                    None, ADD)

            tile_idx = 0
            act_n = 0

            def produce(dst, hq_t, col):
                nonlocal tile_idx, act_n
                i = tile_idx
                tile_idx += 1
                hpb_t = {id(hq01): hpb01, id(hq23): hpb23,
                         id(hq4p): hpb4d}[id(hq_t)]
                on_act = i % act_every == act_every - 1
                if on_act and act_skim:
                    # ACT is the tighter queue; skim every act_skim-th ACT
                    # assignment back to DVE to balance
                    act_n += 1
                    if act_n % act_skim == 0:
                        on_act = False
                if on_act:
                    nc.scalar.activation(dst, hpb_t[:], RELU,
                                         bias=hq_t[:, col:col + 1])
                else:
                    nc.vector.tensor_scalar(dst, hpb_t[:],
                                            hq_t[:, col:col + 1], 0.0,
                                            ADD, MAX)

            # ---- deferred-emission schedule ----
            # Engine queues are in-order; the serial drain/stats/tail
            # chains would head-of-line-block the produce stream at every
            # half/rep boundary.  Emit them a few produce-chunks late, and
            # split the tail at each cross-engine hop (DVE newton -> ACT
            # exp -> DVE res) so neither queue ever stalls at its head:
            # by the time each segment surfaces, its dependency is done.
            banks = {}    # (rep, half) -> (bank01, bank23, bank44)
            osqs = {}     # (rep, half) -> (o01, o23, o44, sq01, sq23, sq44)
            psum5s = {}   # rep -> {half: psum5}
            tails = {}    # rep -> dict of tail tiles

            def mm_chunk(rep, half, g, chunk):
                qoff0 = P * half
                if g == 0 and chunk == 0:
                    banks[(rep, half)] = (
                        mps.tile([P, 2, NP], f32, tag="b01", name="bank01"),
                        mps.tile([P, 2, NP], f32, tag="b23", name="bank23"),
                        mps.tile([P, 2, NP], f32, tag="b44", name="bank44"))
                bank01, bank23, bank44 = banks[(rep, half)]
                # pair01 superbuffer: 4 MMs (u = 4c..4c+3)
                sb01 = sbp.tile([P, 8, NP], bf16, tag="sb01")
                for j, u in enumerate(range(4 * chunk, 4 * chunk + 4)):
                    for c in range(2):
                        qq = qoff0 + 32 * g + 16 * c + u
                        produce(sb01[:, 2 * j + c, :], hq01, qq)
                for j, u in enumerate(range(4 * chunk, 4 * chunk + 4)):
                    nc.tensor.matmul(
                        bank01[32 * g:32 * (g + 1), :, :],
                        wsel_sb[:, 0, u, :],
                        sb01[:, 2 * j:2 * j + 2, :],
                        start=(u == 0), stop=(u == 15),
                        tile_position=(0, 32 * g),
                        skip_group_check=True)
                # pair23
                sb23 = sbp.tile([P, 8, NP], bf16, tag="sb23")
                for j, u in enumerate(range(4 * chunk, 4 * chunk + 4)):
                    for c in range(2):
                        qq = qoff0 + 32 * g + 16 * c + u
                        produce(sb23[:, 2 * j + c, :], hq23, qq)
                for j, u in enumerate(range(4 * chunk, 4 * chunk + 4)):
                    nc.tensor.matmul(
                        bank23[32 * g:32 * (g + 1), :, :],
                        wsel_sb[:, 1, u, :],
                        sb23[:, 2 * j:2 * j + 2, :],
                        start=(u == 0), stop=(u == 15),
                        tile_position=(0, 32 * g),
                        skip_group_check=True)
                # model-4: 8 MMs per g -> on chunks 0 and 2
                if chunk % 2 == 0:
                    vc = chunk // 2
                    sb44 = sbp.tile([P, 8, NP], bf16, tag="sb44")
                    for j, v in enumerate(range(4 * vc, 4 * vc + 4)):
                        for c in range(2):
                            w = 64 * half + 16 * g + 8 * c + v
                            produce(sb44[:, 2 * j + c, :], hq4p, w)
                    for j, v in enumerate(range(4 * vc, 4 * vc + 4)):
                        nc.tensor.matmul(
                            bank44[32 * g:32 * (g + 1), :, :],
                            wsel_sb[:, 2, v, :],
                            sb44[:, 2 * j:2 * j + 2, :],
                            start=(v == 0), stop=(v == 7),
                            tile_position=(0, 32 * g),
                            skip_group_check=True)

            def emit_drains(rep, half):
                bank01, bank23, bank44 = banks[(rep, half)]
                o01 = spool.tile([P, 2, NP], bf16, tag="o01")
                o23 = spool.tile([P, 2, NP], bf16, tag="o23")
                o44 = spool.tile([P, 2, NP], bf16, tag="o44")
                sq01 = spool.tile([P, 2, NP], bf16, tag="sq01")
                sq23 = spool.tile([P, 2, NP], bf16, tag="sq23")
                sq44 = spool.tile([P, 2, NP], bf16, tag="sq44")
                osqs[(rep, half)] = (o01, o23, o44, sq01, sq23, sq44)
                # drains (+b2) all on ACT: the DVE queue is the binding
                # resource; ACT has slack
                nc.scalar.activation(o01[:], bank01[:], IDENT,
                                     bias=b2c_sb[:, 0:1])
                nc.scalar.activation(o23[:], bank23[:], IDENT,
                                     bias=b2c_sb[:, 1:2])
                nc.scalar.activation(o44[:], bank44[:], IDENT,
                                     bias=b2c_sb[:, 2:3])
                # squares: 01/23 on DVE (bf16 TT), 44 on ACT
                nc.vector.tensor_tensor(sq01[:], o01[:], o01[:], MULT)
                nc.vector.tensor_tensor(sq23[:], o23[:], o23[:], MULT)
                nc.scalar.activation(sq44[:], o44[:], SQUARE)

            def emit_stats(rep, half):
                # model sums / sums of squares via PE
                psum5 = mps.tile([P, 2, NP], f32, tag="stats")
                psum5s.setdefault(rep, {})[half] = psum5
                o01, o23, o44, sq01, sq23, sq44 = osqs[(rep, half)]
                for dst, srcs in ((psum5[:, 0, :], (o01, o23, o44)),
                                  (psum5[:, 1, :], (sq01, sq23, sq44))):
                    first = True
                    for c in range(2):
                        nc.tensor.matmul(dst, smat_sb[:, c, :],
                                         srcs[0][:, c, :],
                                         start=first, stop=False)
                        first = False
                        nc.tensor.matmul(dst, smat_sb[:, c, :],
                                         srcs[1][:, c, :],
                                         start=False, stop=False)
                        nc.tensor.matmul(dst, smat_sb[:, 2 + c, :],
                                         srcs[2][:, c, :],
                                         start=False, stop=(c == 1))

            def emit_tail_a(rep):
                # d = ss5 - 0.2*s5^2 per half, then bf16 fast-rsqrt (all
                # DVE; int16/bf16 keeps the 2x packed mode)
                p5 = psum5s[rep]
                d_all = spool.tile([P, 2, NP], bf16, tag="d_all")
                sd = spool.tile([P, 2, NP], bf16, tag="sd")
                tails[rep] = {"p5": p5, "sd": sd}
                for half in range(2):
                    su2 = spool.tile([P, NP], f32, tag="su2")
                    s2a = spool.tile([P, NP], f32, tag="s2a")
                    nc.scalar.activation(su2[:], p5[half][:, 0, :],
                                         IDENT, scale=float(np.sqrt(0.2)))
                    nc.scalar.activation(s2a[:], su2[:], SQUARE)
                    nc.vector.tensor_tensor(d_all[:, half, :],
                                            p5[half][:, 1, :],
                                            s2a[:], SUB)
                ti = spool.tile([P, 2, NP], i16, tag="ti")
                y0i = spool.tile([P, 2, NP], i16, tag="y0i")
                d_i = d_all[:].bitcast(i16)
                nc.vector.tensor_scalar(ti[:], d_i, 1, None, SHR)
                # MAGIC16 - t  ==  (t - MAGIC16) * -1
                nc.vector.tensor_scalar(y0i[:], ti[:], MAGIC16, -1.0,
                                        SUB, MULT)
                y0 = y0i[:].bitcast(bf16)
                nc.vector.tensor_tensor(sd[:], d_all[:], y0, MULT)

            def emit_tail_b(rep):
                # e2 = exp(-0.5*sd + ln(0.2)) = 0.2*exp(-std)   (ACT)
                e2 = spool.tile([P, 2, NP], f32, tag="e2")
                tails[rep]["e2"] = e2
                nc.scalar.activation(e2[:], tails[rep]["sd"][:], EXP,
                                     scale=-0.5, bias=ln02[:, 0:1])

            def emit_tail_c(rep):
                t = tails.pop(rep)
                p5, e2 = t["p5"], t["e2"]
                res_all = spool.tile([P, 2, NP], f32, tag="res_all")
                for half in range(2):
                    nc.vector.tensor_tensor(res_all[:, half, :],
                                            e2[:, half, :],
                                            p5[half][:, 0, :], MULT)
                    nc.sync.dma_start(out[P * half:P * (half + 1), :],
                                      res_all[:, half, :])

            # boundary b = number of mm chunks emitted before the event
            D_DRAIN, D_STATS = 1, 3
            D_TA, D_TB, D_TC = 4, 5, 6
            boundary = {}
            for rep in range(repeat):
                for half in range(2):
                    end = 32 * rep + 16 * half + 16
                    boundary.setdefault(end + D_DRAIN, []).append(
                        ("drain", rep, half))
                    boundary.setdefault(end + D_STATS, []).append(
                        ("stats", rep, half))
                end = 32 * rep + 32
                boundary.setdefault(end + D_TA, []).append(("ta", rep))
                boundary.setdefault(end + D_TB, []).append(("tb", rep))
                boundary.setdefault(end + D_TC, []).append(("tc", rep))
            fns = {"drain": emit_drains, "stats": emit_stats,
                   "ta": emit_tail_a, "tb": emit_tail_b, "tc": emit_tail_c}

            count = 0
            for rep in range(repeat):
                for half in range(2):
                    for g in range(4):
                        for chunk in range(4):
                            mm_chunk(rep, half, g, chunk)
                            count += 1
                            for kind, *a in boundary.pop(count, []):
                                fns[kind](*a)
            for b in sorted(boundary):
                for kind, *a in boundary[b]:
                    fns[kind](*a)

    nc.compile()
    return nc


class _Runner:
    def __init__(self, nc, n_cores=NCORES):
        import jax
        from jax.sharding import Mesh, PartitionSpec
        from jax.experimental.shard_map import shard_map
        import concourse.mybir as mybir
        from concourse import bass2jax

        bass2jax.install_neuronx_cc_hook()
        self.jax = jax
        self.n_cores = n_cores
        self.in_names, self.out_names, out_avals, self.zero_outs = [], [], [], []
        pname = nc.partition_id_tensor.name if nc.partition_id_tensor else None
        for alloc in nc.m.functions[0].allocations:
            if not isinstance(alloc, mybir.MemoryLocationSet):
                continue
            name = alloc.memorylocations[0].name
            if alloc.kind == "ExternalInput":
                if name != pname:
                    self.in_names.append(name)
            elif alloc.kind == "ExternalOutput":
                self.out_names.append(name)
                shape = tuple(alloc.tensor_shape)
                dtype = mybir.dt.np(alloc.dtype)
                out_avals.append(jax.core.ShapedArray(shape, dtype))
                self.zero_outs.append(np.zeros(shape, dtype))
        n_params = len(self.in_names)
        n_outs = len(out_avals)
        all_names = tuple(self.in_names + self.out_names + ([pname] if pname else []))
        out_names = tuple(self.out_names)

        def _body(*args):
            operands = list(args)
            if pname is not None:
                operands.append(bass2jax.partition_id_tensor())
            return tuple(bass2jax._bass_exec_p.bind(
                *operands, out_avals=tuple(out_avals), in_names=all_names,
                out_names=out_names, lowering_input_output_aliases=(),
                sim_require_finite=True, sim_require_nnan=True, nc=nc))

        devices = jax.devices()[:n_cores]
        mesh = Mesh(np.asarray(devices), ("core",))
        self.fn = jax.jit(
            shard_map(_body, mesh=mesh,
                      in_specs=(PartitionSpec("core"),) * (n_params + n_outs),
                      out_specs=(PartitionSpec("core"),) * n_outs,
                      check_rep=False),
            keep_unused=True)

    def concat_inputs(self, in_maps):
        cat = [np.concatenate([np.asarray(m[name]) for m in in_maps], axis=0)
               for name in self.in_names]
        cat += [np.zeros((self.n_cores * z.shape[0], *z.shape[1:]), z.dtype)
                for z in self.zero_outs]
        return cat

    def run(self, in_maps):
        outs = self.fn(*self.concat_inputs(in_maps))
        self.jax.block_until_ready(outs)
        res = []
        for c in range(self.n_cores):
            d = {}
            for i, name in enumerate(self.out_names):
                full = np.asarray(outs[i])
                per = full.shape[0] // self.n_cores
                d[name] = full[c * per:(c + 1) * per]
            res.append(d)
        return res

    def time_it(self, in_maps, iters=20):
        args = [self.jax.device_put(x) for x in self.concat_inputs(in_maps)]
        outs = self.fn(*args)
        self.jax.block_until_ready(outs)
        times = []
        for _ in range(iters):
            t0 = time.perf_counter()
            outs = self.fn(*args)
            self.jax.block_until_ready(outs)
            times.append((time.perf_counter() - t0) * 1e9)
        times.sort()
        return times


def _host_prep(query_features, prototypes, W1, b1, W2, b2):
    """Host-side layout prep; returns per-core in_maps."""
    q = np.asarray(query_features, np.float32)
    p = np.asarray(prototypes, np.float32)
    W1 = np.asarray(W1, np.float32)
    b1 = np.asarray(b1, np.float32)
    W2 = np.asarray(W2, np.float32)
    b2 = np.asarray(b2, np.float32)

    absW2 = np.abs(W2)                       # [M, H]
    sgnW2 = np.where(W2 >= 0, 1.0, -1.0).astype(np.float32)
    W1q = W1[:, :, :EMB] * absW2[:, :, None]  # [M, H, E]
    W1p = W1[:, :, EMB:] * absW2[:, :, None]
    b1s = b1 * absW2                         # [M, H]

    qT = np.ascontiguousarray(q.T)           # [E, NQ]
    pT = np.ascontiguousarray(p.T)           # [E, NP]
    w1qT = np.ascontiguousarray(W1q.transpose(2, 0, 1).reshape(EMB, NM * HID))
    w1pT = np.ascontiguousarray(W1p.transpose(2, 0, 1).reshape(EMB, NM * HID))

    b1c = np.zeros((P, 3), np.float32)
    b1c[:HID, 0], b1c[HID:, 0] = b1s[0], b1s[1]
    b1c[:HID, 1], b1c[HID:, 1] = b1s[2], b1s[3]
    b1c[:HID, 2], b1c[HID:, 2] = b1s[4], b1s[4]

    wsel = np.zeros((P, 3, 16, 32), np.float32)
    model_pairs = ((0, 1), (2, 3), (4, 4))
    for pi, (ma, mb) in enumerate(model_pairs):
        for u in range(16):
            wsel[:HID, pi, u, 2 * u] = sgnW2[ma]
            wsel[HID:, pi, u, 2 * u + 1] = sgnW2[mb]
    wsel = wsel.reshape(P, 3 * 16 * 32).astype(ml_dtypes.bfloat16)

    b2c = np.zeros((P, 3), np.float32)
    b2c[0::2, 0], b2c[1::2, 0] = b2[0], b2[1]
    b2c[0::2, 1], b2c[1::2, 1] = b2[2], b2[3]
    b2c[:, 2] = b2[4]

    # smat: [P01_c0 | P01_c1 | P44_c0 | P44_c1], each [128, 128]
    smat = np.zeros((P, 4, P), np.float32)
    for c in range(2):
        for g in range(4):
            for u in range(16):
                for m in range(2):
                    smat[32 * g + 2 * u + m, c, 32 * g + 16 * c + u] = 1.0
            for v in range(8):
                for par in range(2):
                    smat[32 * g + 2 * v + par, 2 + c,
                         32 * g + 16 * c + 2 * v + par] = 1.0
    smat = smat.reshape(P, 4 * P).astype(ml_dtypes.bfloat16)

    shared = dict(pT=pT, w1qT=w1qT, w1pT=w1pT, b1c=b1c, wsel=wsel,
                  b2c=b2c, smat=smat)
    in_maps = []
    for c in range(NCORES):
        m = dict(shared)
        m["qT"] = np.ascontiguousarray(qT[:, c * NQL:(c + 1) * NQL])
        in_maps.append(m)
    return in_maps


def _get_runner(repeat=1, act_every=3, tail_mode="newton"):
    key = (repeat, act_every, tail_mode)
    with _LOCK:
        if key not in _CACHE:
            nc = _build_nc(repeat=repeat, act_every=act_every,
                           tail_mode=tail_mode)
            _CACHE[key] = _Runner(nc)
        return _CACHE[key]


def kernel(query_features, prototypes, W1, b1, W2, b2):
    in_maps = _host_prep(query_features, prototypes, W1, b1, W2, b2)
    runner = _get_runner()
    res = runner.run(in_maps)
    return np.concatenate([res[c]["out"] for c in range(NCORES)], axis=0)

